# revision 64
# baseline (speedup 1.0000x reference)
"""Trainium2 Bass kernel for KosmosTextAttention (B=2, S=2048, E=2048, H=32).

Sharding: launch 1 = tensor-parallel over heads (4 groups) x data-parallel
over batch (2) -> 8 cores; launch 2 = row-parallel out-proj (LN folded to
host glue between launches).

Launch 1 per core (b, g), software-pipelined attention over (s-block,
head-pair) pairs:
  all q/k/v projections form a filler stream of atomic 17-op groups
  (16 matmuls + bias-evict) drained lazily (fill_until) at each
  producer's last-needed point, so the PE stays saturated while the ACT
  exp stream runs concurrently.  Pair order: sb 0 bootstrap, ACT-heavy
  sb 3/2 interleaved midway (plentiful filler), filler-surplus sb 1 and
  sb 0 remnants close the schedule PE-bound.
  Attention per (sb, head-pair j): per t-tile: scores matmul pair
  (K=64, head pairs packed at base partitions 0/64) restricted to the
  causally valid column range -> exp on ACT -> diagonal tiles multiplied
  by a 0/1 band (DVE, bf16) -> transposed PV: 8 matmuls (one per
  head x s-subtile) accumulate ctx[s,d] with full 128-partition outputs
  (ones column in v gives row sums in col D; one accumulation group per
  2KB PSUM bank).  After the t-loop: DVE reciprocal of the sums + one
  broadcast tensor_tensor normalize into an [s, channel] staging tile;
  per-pair DMA writes ctx[S, ES] (v-bias folded into the v projection).
Host: LayerNorm only (gamma/beta folded into W2/b2), reshard rows.
Launch 2 per core (512 rows): out = yT.T @ W2 with W2 = gamma*Wo.T
(bf16); bias b2 = beta@Wo.T+bo added on host.
"""

import numpy as np
from ml_dtypes import bfloat16

import concourse.bass as bass
import concourse.mybir as mybir
import concourse.tile as tile
from concourse import bacc
from concourse.bass_utils import run_bass_kernel_spmd

B, S, E, H = 2, 2048, 2048, 32
D = 64
G = 4            # head groups
HG = H // G      # 8 heads per group
ES = E // G      # 512 channels per group
SCALE = D ** -0.5
LN_EPS = 1e-5
P = 128
FD = 512         # matmul free dim / s-block
NK = E // P      # 16 contraction tiles
NT = S // P      # 16 t tiles
NSB = S // FD    # 4 s blocks
NM = ES // P     # 4 output-channel tiles per group
CW = D + 1       # 65: v columns per head incl. ones column
NP = HG // 2     # 4 head pairs
f32 = mybir.dt.float32
bf16 = mybir.dt.bfloat16
AF = mybir.ActivationFunctionType
ALU = mybir.AluOpType


def build_launch1():
    nc = bacc.Bacc(None, target_bir_lowering=False)
    hsT = nc.declare_dram_parameter("hsT", [E, S], bf16, isOutput=False)
    wqT = nc.declare_dram_parameter("wqT", [E, ES], bf16, isOutput=False)
    wkT = nc.declare_dram_parameter("wkT", [E, ES], bf16, isOutput=False)
    wvT = nc.declare_dram_parameter("wvT", [E, ES], bf16, isOutput=False)
    bqk = nc.declare_dram_parameter("bqk", [2, ES], f32, isOutput=False)
    bvr = nc.declare_dram_parameter("bvr", [1, ES], f32, isOutput=False)
    band = nc.declare_dram_parameter("band", [NSB * P, 2 * FD], bf16,
                                     isOutput=False)
    ctx = nc.declare_dram_parameter("ctx", [S, ES], bf16, isOutput=True)

    hsT_r = hsT.rearrange("(ko p) s -> p ko s", p=P)
    wq_r = wqT.rearrange("(ko p) m -> p ko m", p=P)
    wk_r = wkT.rearrange("(ko p) m -> p ko m", p=P)
    wv_r = wvT.rearrange("(ko p) m -> p ko m", p=P)
    ctx_r = ctx.rearrange("(o p) c -> p o c", p=P)

    with tile.TileContext(nc) as tc:
        with (
            tc.tile_pool(name="res", bufs=1) as res,
            tc.tile_pool(name="const", bufs=1) as const,
            tc.tile_pool(name="pt_pool", bufs=3) as pt_pool,
            tc.tile_pool(name="rec_pool", bufs=2) as rec_pool,
            tc.tile_pool(name="cstage", bufs=2) as cstage,
            tc.tile_pool(name="ppsum", bufs=2, space="PSUM") as ppsum,
            tc.tile_pool(name="sc_psum", bufs=2, space="PSUM") as sc_psum,
            tc.tile_pool(name="pv_psum", bufs=1, space="PSUM") as pv_psum,
        ):
            # ---- resident tensors ----
            hs_sb = res.tile([P, NK, S], bf16)
            wq_sb = res.tile([P, NK, ES], bf16)
            wk_sb = res.tile([P, NK, ES], bf16)
            wv_sb = res.tile([P, NK, ES], bf16)
            qT_sb = res.tile([P, NM, S], bf16)
            kT_sb = res.tile([P, NM, S], bf16)
            v_sb = res.tile([P, NT, HG * CW], bf16)

            bias_sb = const.tile([P, 2, NM], f32)
            bvrep = const.tile([P, ES], f32)
            band_sb = const.tile([P, NSB, 2, FD], bf16)
            nc.any.memset(v_sb[:, :, D::CW], 1.0)

            # PE warm-up: dummy matmuls fill the DMA-gated startup window so
            # the p-state ramp completes before the first k-proj data lands.
            warm = const.tile([P, 2 * P], bf16, name="warm")
            nc.vector.memset(warm, 0.0)
            wps = ppsum.tile([P, FD], f32, name="warm_ps", tag="proj")
            for _ in range(12):
                nc.tensor.matmul(wps[:, 0 : 2 * P], lhsT=warm[:, 0:P],
                                 rhs=warm, start=True, stop=True)

            # ---- DMA plan: single serial bus, exact need order.
            # k(0,*) needs hs(sb0)+wk; v(0..3) needs wv+bvrep; q(0,*) needs
            # wq; attention sb0 needs band+bqk; later sb need hs(sb).
            dma = nc.sync.dma_start
            chunks = [(0, 1), (1, 2), (2, 4), (4, 6), (6, 8), (8, 10),
                      (10, 12), (12, 14), (14, 16)]
            for c0, c1 in chunks:
                dma(hs_sb[:, c0:c1, 0:FD], hsT_r[:, c0:c1, 0:FD])
                dma(wk_sb[:, c0:c1, :], wk_r[:, c0:c1, :])
                if c0 == 0:
                    dma(bias_sb, bqk.rearrange("w (mo p) -> p w mo", p=P))
            for c in range(0, NK, 2):
                dma(wv_sb[:, c : c + 2, :], wv_r[:, c : c + 2, :])
            dma(bvrep, bvr[0:1, :].to_broadcast([P, ES]))
            dma(band_sb, band.rearrange("(o p) (i f) -> p o i f", p=P, i=2))
            for c in range(0, NK, 4):
                dma(wq_sb[:, c : c + 4, :], wq_r[:, c : c + 4, :])
            for sbk in range(1, NSB):
                ssl = slice(sbk * FD, (sbk + 1) * FD)
                dma(hs_sb[:, 0:8, ssl], hsT_r[:, 0:8, ssl])
                dma(hs_sb[:, 8:16, ssl], hsT_r[:, 8:16, ssl])

            # ---- projection filler: per-op closures (16 matmuls + evict)
            # drained at instruction granularity.  Accumulation groups live
            # in their own PSUM banks, so foreign matmuls may interleave
            # between a group's members (2KB zero-region rule). ----
            def qk_ops(w_idx, w_sb, dest, sbk, m, ps_ap=None):
                ssl = slice(sbk * FD, (sbk + 1) * FD)
                box = [None]

                def mk(k):
                    def op():
                        if k == 0:
                            box[0] = (ps_ap if ps_ap is not None else
                                      ppsum.tile([P, FD], f32,
                                                 name="proj_ps", tag="proj"))
                        nc.tensor.matmul(
                            box[0],
                            lhsT=w_sb[:, k, m * P : (m + 1) * P],
                            rhs=hs_sb[:, k, ssl],
                            start=(k == 0),
                            stop=(k == NK - 1),
                        )
                    return op

                def ev():
                    nc.vector.tensor_scalar(
                        out=dest[:, m, ssl], in0=box[0],
                        scalar1=bias_sb[:, w_idx, m : m + 1],
                        scalar2=None, op0=ALU.add,
                    )
                return [mk(k) for k in range(NK)] + [ev]

            def v_ops(so, ps_ap=None):
                box = [None]

                def mk(k):
                    def op():
                        if k == 0:
                            box[0] = (ps_ap if ps_ap is not None else
                                      ppsum.tile([P, ES], f32,
                                                 name="v_ps", tag="proj"))
                        nc.tensor.matmul(
                            box[0],
                            lhsT=hs_sb[:, k, so * P : (so + 1) * P],
                            rhs=wv_sb[:, k, :],
                            start=(k == 0),
                            stop=(k == NK - 1),
                        )
                    return op

                def ev():
                    # strided evict (+v bias, commutes through the softmax
                    # average)
                    nc.vector.tensor_tensor(
                        v_sb[:, so, :].rearrange(
                            "p (h w) -> p h w", w=CW)[:, :, 0:D],
                        box[0].rearrange("p (h w) -> p h w", w=D),
                        bvrep.rearrange("p (h w) -> p h w", w=D),
                        ALU.add,
                    )
                return [mk(k) for k in range(NK)] + [ev]

            def filler_stream():
                # Round 0: two k-groups zipped per k-step so every arriving
                # hs/wk DMA chunk unlocks 2 matmuls (keeps the DMA-gated
                # startup window PE-busy).  Later rounds latest-allowed;
                # k(3,m) deferred past q so it can drain inside its consumer
                # pair's t-loop; q(1,1..3)/q(0,2..3) are the tail filler.
                def kg(sbk, m):
                    return ("k", sbk, m), qk_ops(1, wk_sb, kT_sb, sbk, m)

                def qg(sbk, m):
                    return ("q", sbk, m), qk_ops(0, wq_sb, qT_sb, sbk, m)

                def vg(so):
                    return ("v", so, 0), v_ops(so)

                def one(kv):
                    key, ops = kv
                    for op in ops[:-1]:
                        yield None, op
                    yield key, ops[-1]

                def zip2(kv1, kv2):
                    (k1, o1), (k2, o2) = kv1, kv2
                    for i in range(max(len(o1), len(o2))):
                        if i < len(o1):
                            yield (k1 if i == len(o1) - 1 else None), o1[i]
                        if i < len(o2):
                            yield (k2 if i == len(o2) - 1 else None), o2[i]

                def zipn(*kvs):
                    n = max(len(o) for _, o in kvs)
                    for i in range(n):
                        for key, ops in kvs:
                            if i < len(ops):
                                yield (key if i == len(ops) - 1 else
                                       None), ops[i]

                # startup is DMA-gated: borrow 2 idle sc_psum banks so FOUR
                # k-groups (then v-groups) advance per arriving hs/wk chunk,
                # keeping the PE busy at the bus rate
                scb = sc_psum.tile([P, 2, FD], f32, tag="sc")
                yield from zipn(
                    kg(0, 0), kg(0, 1),
                    (("k", 0, 2), qk_ops(1, wk_sb, kT_sb, 0, 2,
                                         ps_ap=scb[:, 0, :])),
                    (("k", 0, 3), qk_ops(1, wk_sb, kT_sb, 0, 3,
                                         ps_ap=scb[:, 1, :])),
                )
                scb2 = sc_psum.tile([P, 2, FD], f32, tag="sc")
                yield from zipn(
                    vg(0), vg(1),
                    (("v", 2, 0), v_ops(2, ps_ap=scb2[:, 0, :])),
                    (("v", 3, 0), v_ops(3, ps_ap=scb2[:, 1, :])),
                )
                yield from zipn(qg(0, 0), qg(0, 1))
                yield from one(kg(1, 0))
                yield from one(qg(1, 0))
                for so in range(4, 8):
                    yield from one(vg(so))
                yield from one(kg(2, 0))
                yield from one(qg(3, 0))
                for so in range(8, 16):
                    yield from one(vg(so))
                yield from one(qg(2, 0))
                yield from one(kg(3, 0))
                for m in range(1, NM):
                    yield from one(kg(1, m))
                    yield from one(kg(2, m))
                    yield from one(qg(3, m))
                    yield from one(qg(2, m))
                    yield from one(kg(3, m))
                # tail reserve: only fill_until may cross this marker, so
                # mid-kernel pacing can't drain the closing pairs' filler
                yield "RESERVE", None
                for m in range(1, NM):
                    yield from one(qg(1, m))
                yield from one(qg(0, 2))
                yield from one(qg(0, 3))

            filler = filler_stream()
            fill_done = set()
            pend = [None]

            def fill_some(n, cross_reserve=False):
                for _ in range(n):
                    item = pend[0]
                    pend[0] = None
                    if item is None:
                        item = next(filler, None)
                    if item is None:
                        return False
                    key, op = item
                    if op is None:  # RESERVE marker
                        if not cross_reserve:
                            pend[0] = item
                            return False
                        continue
                    op()
                    if key is not None:
                        fill_done.add(key)
                return True

            def fill_until(key):
                while key not in fill_done:
                    if not fill_some(1, cross_reserve=True):
                        return

            def fill_toward(key, n):
                # paced partial drain toward `key`: at most n ops, stop as
                # soon as the key completes
                for _ in range(n):
                    if key in fill_done:
                        return
                    if not fill_some(1, cross_reserve=True):
                        return

            # ---- attention main loop.  Order: sb 0 (startup, DMA-gated),
            # (1,0) (pulls v(4..7)), the ACT-heavy sb 3/2 interleaved midway
            # where filler (v(8..15), k/q of rounds 2,3) is plentiful, and
            # the filler-surplus sb=1 pairs last so the kernel ends
            # PE-bound instead of starving behind the exp stream. ----
            pairs = [(0, 0), (0, 1),
                     (1, 0), (3, 0), (2, 0), (3, 1), (2, 1),
                     (3, 2), (2, 2), (3, 3), (2, 3),
                     (1, 1), (1, 2), (1, 3), (0, 2), (0, 3)]
            for pi, (sb, j) in enumerate(pairs):
                ntv = 4 * sb + 4  # causally-valid t tiles
                fill_until(("q", sb, j))
                if 1:  # pair body
                    pv = pv_psum.tile([P, 2 * NM, P], f32, tag="pv")
                    for t in range(ntv):
                        o = t - 4 * sb
                        lo = max(0, P * o)  # first valid col in s-block
                        fill_until(("v", t, 0))
                        # k-tile needed now, plus 2-tile lookahead so a
                        # deferred k-group's bias-evict latency is hidden
                        fill_until(("k", t // 4, j))
                        fill_until(("k", min(sb, (t + 2) // 4), j))
                        if t == max(0, ntv - 4) and pi + 1 < len(pairs):
                            # prefetch next pair's k/q early enough that the
                            # bias-evict clears the DVE queue before the
                            # pair boundary
                            nsb_, nj_ = pairs[pi + 1]
                            fill_until(("q", nsb_, nj_))
                        # build-time dependency audit: producers must have
                        # been emitted before their consumers
                        assert ("k", t // 4, j) in fill_done, (sb, j, t)
                        assert ("v", t, 0) in fill_done, (sb, j, t)
                        assert ("q", sb, j) in fill_done, (sb, j, t)
                        sc = sc_psum.tile([P, 2, FD], f32, tag="sc")
                        for i in range(2):
                            b0 = D * i
                            nc.tensor.matmul(
                                sc[:, i, lo:FD],
                                lhsT=kT_sb[b0 : b0 + D, j, t * P : (t + 1) * P],
                                rhs=qT_sb[b0 : b0 + D, j,
                                          sb * FD + lo : (sb + 1) * FD],
                                start=True,
                                stop=True,
                            )
                        pt = pt_pool.tile([P, 2, FD], bf16, tag="pt")
                        nc.scalar.activation(pt[:, :, lo:FD], sc[:, :, lo:FD],
                                             AF.Exp)
                        if o >= 0:  # diagonal tile: zero the masked region
                            nc.vector.tensor_tensor(
                                pt[:, :, lo:FD], pt[:, :, lo:FD],
                                band_sb[:, o, :, lo:FD], ALU.mult,
                            )
                        # One accumulation group per PSUM bank (slots i*NM..
                        # i*NM+3 share a 2KB zero region): start marks the
                        # whole bank pending-zero, each slot's first write
                        # clears its own bytes, stop on the bank's last
                        # matmul.
                        for i in range(2):
                            h = 2 * j + i
                            for st in range(max(0, o), NM):
                                nc.tensor.matmul(
                                    pv[:, i * NM + st, 0:CW],
                                    lhsT=pt[:, i, st * P : (st + 1) * P],
                                    rhs=v_sb[:, t, h * CW : (h + 1) * CW],
                                    start=(t == 0 and st == 0),
                                    stop=(t == 4 * sb + 3 and st == NM - 1),
                                )
                        # pace ~2 filler matmuls per t-iter to spread pull
                        # lumps; off for the closing pairs so their reserved
                        # tail filler isn't drained early
                        if pi < 11:
                            fill_some(2)
                    # evict: normalize by row sums into [s, channel] staging
                    rec = rec_pool.tile([P, 2 * NM, 1], f32, tag="rec")
                    nc.vector.reciprocal(rec, pv[:, :, D : D + 1])
                    if pi < len(pairs) - 1:
                        stage = cstage.tile([P, NM, P], bf16, tag="stage")
                        nc.vector.tensor_tensor(
                            stage.rearrange("p st (i d) -> p i st d", i=2),
                            pv[:, :, 0:D].rearrange(
                                "p (i st) d -> p i st d", i=2),
                            rec[:, :, 0:1].to_broadcast(
                                [P, 2 * NM, D]).rearrange(
                                "p (i st) d -> p i st d", i=2),
                            ALU.mult,
                        )
                        nc.sync.dma_start(
                            ctx_r[:, sb * NM : (sb + 1) * NM,
                                  j * P : (j + 1) * P],
                            stage,
                        )
                    else:
                        # last pair: evict per bank so the final drain chain
                        # (normalize -> DMA -> sem) is as short as possible
                        for i in range(2):
                            stg = cstage.tile([P, NM, D], bf16,
                                              name=f"stgL{i}", tag=f"stgL{i}")
                            nc.vector.tensor_tensor(
                                stg,
                                pv[:, i * NM : (i + 1) * NM, 0:D],
                                rec[:, i * NM : (i + 1) * NM, 0:1]
                                .to_broadcast([P, NM, D]),
                                ALU.mult,
                            )
                            nc.sync.dma_start(
                                ctx_r[:, sb * NM : (sb + 1) * NM,
                                      j * P + i * D : j * P + (i + 1) * D],
                                stg,
                            )
            while fill_some(1):
                pass
    nc.compile()
    return nc


def build_launch2():
    RPC = B * S // 8  # 512 rows per core
    NMT = RPC // P    # 4 row tiles
    NNT = E // FD     # 4 out-column tiles
    nc = bacc.Bacc(None, target_bir_lowering=False)
    xT = nc.declare_dram_parameter("xT", [E, RPC], bf16, isOutput=False)
    w2 = nc.declare_dram_parameter("w2", [E, E], bf16, isOutput=False)
    outr = nc.declare_dram_parameter("outr", [RPC, E], f32, isOutput=True)

    w2_r = w2.rearrange("(ko p) e -> p ko e", p=P)
    xT_r = xT.rearrange("(ko p) r -> p ko r", p=P)
    out_r = outr.rearrange("(mo p) e -> p mo e", p=P)

    with tile.TileContext(nc) as tc:
        with (
            tc.tile_pool(name="xp", bufs=1) as xp,
            tc.tile_pool(name="w2p", bufs=2) as w2p,
            tc.tile_pool(name="ostage", bufs=6) as ostage,
            tc.tile_pool(name="opsum", bufs=8, space="PSUM") as opsum,
        ):
            x_sb = xp.tile([P, NK, RPC], bf16)
            # PE warm-up: dummy matmuls fill the DMA-gated startup window so
            # the p-state ramp (0.65 -> 1.2 -> 2.4 GHz after 3us continuous
            # execution) completes before real data lands.
            warm = xp.tile([P, 2 * P], bf16, name="warm")
            nc.vector.memset(warm, 0.0)
            wps = opsum.tile([P, 2 * P], f32, name="warm_ps", tag="ops")
            for _ in range(12):
                nc.tensor.matmul(wps, lhsT=warm[:, 0:P], rhs=warm,
                                 start=True, stop=True)
            w_sbs = [
                w2p.tile([P, NK, FD], bf16, name=f"w2_{nt}", tag=f"w2_{nt}",
                         bufs=1)
                for nt in range(NNT)
            ]
            # serial-DMA need order: w2(nt0) / x interleaved, then w2 nt1-3
            nc.sync.dma_start(w_sbs[0][:, 0:1, :], w2_r[:, 0:1, 0:FD])
            nc.sync.dma_start(x_sb[:, 0:1, :], xT_r[:, 0:1, :])
            for c in range(1, 8):
                nc.sync.dma_start(
                    w_sbs[0][:, 2 * c - 1 : 2 * c + 1, :],
                    w2_r[:, 2 * c - 1 : 2 * c + 1, 0:FD],
                )
                nc.sync.dma_start(
                    x_sb[:, 2 * c - 1 : 2 * c + 1, :],
                    xT_r[:, 2 * c - 1 : 2 * c + 1, :],
                )
            nc.sync.dma_start(w_sbs[0][:, 15:16, :], w2_r[:, 15:16, 0:FD])
            nc.sync.dma_start(x_sb[:, 15:16, :], xT_r[:, 15:16, :])
            for nt in range(1, NNT):
                for c in range(2):
                    nc.sync.dma_start(
                        w_sbs[nt][:, 8 * c : 8 * c + 8, :],
                        w2_r[:, 8 * c : 8 * c + 8, nt * FD : (nt + 1) * FD],
                    )
            for nt in range(NNT):
                w_sb = w_sbs[nt]
                for mt in range(NMT):
                    # split the very last group into two 256-col halves so
                    # the closing evict->DMA->sem chain is half-sized
                    cols = ([(0, FD)] if not (nt == NNT - 1 and mt == NMT - 1)
                            else [(0, FD // 2), (FD // 2, FD)])
                    for c0, c1 in cols:
                        ps = opsum.tile([P, FD], f32, tag="ops")
                        for k in range(NK):
                            nc.tensor.matmul(
                                ps[:, 0 : c1 - c0],
                                lhsT=x_sb[:, k, mt * P : (mt + 1) * P],
                                rhs=w_sb[:, k, c0:c1],
                                start=(k == 0),
                                stop=(k == NK - 1),
                            )
                        ost = ostage.tile([P, FD], f32, tag="ost")
                        nc.any.tensor_copy(out=ost[:, 0 : c1 - c0],
                                           in_=ps[:, 0 : c1 - c0])
                        nc.sync.dma_start(
                            out_r[:, mt, nt * FD + c0 : nt * FD + c1],
                            ost[:, 0 : c1 - c0],
                        )
    nc.compile()
    return nc


def _make_band():
    """band[o*128+tt, i*512+ss] = 1.0 if ss >= tt + 128*o else 0 (i = 0, 1)."""
    tt = np.arange(P)[:, None]
    ss = np.arange(FD)[None, :]
    bands = [(ss >= tt + P * o).astype(np.float32) for o in range(NSB)]
    b = np.stack(bands)                      # [4, 128, 512]
    b = np.tile(b, (1, 1, 2))                # [4, 128, 1024]
    return b.reshape(NSB * P, 2 * FD).astype(bfloat16)


def _prep_launch1_inputs(hidden_states, Wq, bq, Wk, bk, Wv, bv):
    hsT = [np.ascontiguousarray(hidden_states[b].T).astype(bfloat16)
           for b in range(B)]
    band = _make_band()
    in_maps = []
    for c in range(8):
        b, g = c // G, c % G
        sl = slice(g * ES, (g + 1) * ES)
        in_maps.append({
            "hsT": hsT[b],
            "wqT": np.ascontiguousarray(Wq[sl, :].T * SCALE).astype(bfloat16),
            "wkT": np.ascontiguousarray(Wk[sl, :].T).astype(bfloat16),
            "wvT": np.ascontiguousarray(Wv[sl, :].T).astype(bfloat16),
            "bqk": np.ascontiguousarray(np.stack([bq[sl] * SCALE, bk[sl]])),
            "bvr": np.ascontiguousarray(bv[sl][None, :]),
            "band": band,
        })
    return in_maps


def _assemble_ctx(results1):
    """results1[c]["ctx"] [2048, 512] bf16 -> full ctx [B*S, E] f32."""
    ctx = np.empty((B * S, E), dtype=np.float32)
    for c in range(8):
        b, g = c // G, c % G
        ctx[b * S : (b + 1) * S, g * ES : (g + 1) * ES] = np.asarray(
            results1[c]["ctx"], dtype=np.float32
        )
    return ctx


def run_pipeline(inputs, trace=False):
    hidden_states = np.asarray(inputs["hidden_states"], dtype=np.float32)
    Wq = np.asarray(inputs["Wq"], dtype=np.float32)
    Wk = np.asarray(inputs["Wk"], dtype=np.float32)
    Wv = np.asarray(inputs["Wv"], dtype=np.float32)
    Wo = np.asarray(inputs["Wo"], dtype=np.float32)
    bq = np.asarray(inputs["bq"], dtype=np.float32)
    bk = np.asarray(inputs["bk"], dtype=np.float32)
    bv = np.asarray(inputs["bv"], dtype=np.float32)
    bo = np.asarray(inputs["bo"], dtype=np.float32)
    ln_gamma = np.asarray(inputs["ln_gamma"], dtype=np.float32)
    ln_beta = np.asarray(inputs["ln_beta"], dtype=np.float32)

    core_ids = list(range(8))
    nc1 = build_launch1()
    in_maps1 = _prep_launch1_inputs(hidden_states, Wq, bq, Wk, bk, Wv, bv)
    res1 = run_bass_kernel_spmd(nc1, in_maps1, core_ids, trace=trace)
    ctx = _assemble_ctx(res1.results)

    # host glue: LayerNorm normalize (gamma/beta folded into W2/b2)
    mu = ctx.mean(axis=1, keepdims=True)
    var = ctx.var(axis=1, keepdims=True)
    y = (ctx - mu) / np.sqrt(var + LN_EPS)

    w2 = np.ascontiguousarray(ln_gamma[:, None] * Wo.T).astype(bfloat16)
    b2 = np.ascontiguousarray(ln_beta @ Wo.T + bo)
    RPC = B * S // 8
    nc2 = build_launch2()
    in_maps2 = [
        {
            "xT": np.ascontiguousarray(
                y[c * RPC : (c + 1) * RPC].T
            ).astype(bfloat16),
            "w2": w2,
        }
        for c in range(8)
    ]
    res2 = run_bass_kernel_spmd(nc2, in_maps2, core_ids, trace=trace)
    out = np.concatenate([res2.results[c]["outr"] for c in range(8)], axis=0)
    out = (out + b2[None, :]).reshape(B, S, E)
    ns = None
    if trace:
        parts = [r.exec_time_ns for r in (res1, res2)]
        if all(p is not None for p in parts):
            ns = sum(parts)
    return out, ns, (res1, res2)


def kernel(**inputs):
    out, _, _ = run_pipeline(inputs, trace=False)
    return out
